# revision 1
# baseline (speedup 1.0000x reference)
"""Trainium2 Bass kernel for a 2-layer GCN + MLP head (nn_GCN2).

Strategy (8 NeuronCores, SPMD):
  - Nodes are sharded into 8 destination slabs of 12500.
  - Per GCN layer: each core computes its slab of support = x @ W on the
    TensorEngine (bf16), an AllGather builds the full [100000, 256] bf16
    support table in DRAM, then the SpMM (gather - scale - scatter-add)
    runs as per-edge dma_gather (512B rows) + a one-hot matmul on the
    TensorEngine that performs the val-scaling and segment-sum into PSUM,
    one 128-destination tile at a time.
  - Gathers round-robin over 4 SWDGE queues (spreads descriptor rings
    across Q7 core-pairs; note GPSIMD still executes instructions
    cluster-serially, so per-edge descriptor generation ~7.4ns/slot is
    the span-owning bottleneck).  Do NOT pad with -1 indices: the
    stripping path hangs the device (empty-bucket num_idxs=0 case).
  - Biases fold into the segment-sum PSUM accumulation as a K=128
    matmul (ones in partition 0 of lhsT x bias row in partition 0 of
    rhs), so PSUM evacuation is a single ACT-engine activation
    (Relu / Copy) instead of DVE add+max.
  - The MLP head chains transposed (feature-on-partition) so fc1_W/fc2_W
    serve as lhsT directly; log_softmax over 2 classes runs in batched
    strips using only ACT table-6 functions (Abs/Exp/Ln/Relu/Identity,
    no table switching): a = ln(1+exp(-|d|)), out0 = -relu(d)-a,
    out1 = out0 + d.
"""
import math
import sys

import numpy as np

for _p in ("/opt/trn_rl_repo",):
    if _p not in sys.path:
        sys.path.insert(0, _p)

import ml_dtypes  # noqa: E402
from concourse import bacc, bass, mybir  # noqa: E402
import concourse.tile as tile  # noqa: E402
from concourse.masks import make_identity  # noqa: E402
from concourse.bass_utils import run_bass_kernel_spmd  # noqa: E402

BF16 = mybir.dt.bfloat16
F32 = mybir.dt.float32
I16 = mybir.dt.int16
I32 = mybir.dt.int32
BF = ml_dtypes.bfloat16

N_CORES = 8
N_NODES = 100000
SLAB = N_NODES // N_CORES          # 12500 dest nodes per core
TILE = 128
NT = math.ceil(SLAB / TILE)        # 98 dest tiles (last has 84 rows)
IN_DIM = 512
HID = 256
CH_ROWS = 25000                    # gather-table chunk (int16 index limit)
N_CH = N_NODES // CH_ROWS          # 4
NB = NT * N_CH                     # 392 buckets per core
PIECE_SLOTS = 24576                # idx streaming piece budget (1536 cols)
N_QUEUES = 4                       # SWDGE queues (Q7 core pairs)
GATH_BUFS = 6
HEAD_TILES = 8                     # head tiles per strip (1024 cols)


def _rows_of_tile(t):
    return min(TILE, SLAB - t * TILE)


def preprocess_edges(edge_rows, edge_cols, edge_vals):
    """Sort/bucket edges per core; pad buckets uniformly across cores.

    Returns (G, offs, S, per_core) where bucket b = t * N_CH + ch holds
    G[b] groups of 128 edge-slots; unused trailing slots get index -1
    (the gather ucode skips them).
    """
    rows = edge_rows.astype(np.int64)
    cols = edge_cols.astype(np.int64)
    vals = np.asarray(edge_vals, np.float32)
    core = rows // SLAB

    per_core_sorted = []
    counts = np.zeros((N_CORES, NB), np.int64)
    for p in range(N_CORES):
        sel = np.nonzero(core == p)[0]
        r = rows[sel] - p * SLAB
        c = cols[sel]
        v = vals[sel]
        t = r // TILE
        ch = c // CH_ROWS
        b = t * N_CH + ch
        o = np.argsort(b, kind="stable")
        b_s = b[o]
        counts[p] = np.bincount(b_s, minlength=NB)
        per_core_sorted.append((b_s, (r % TILE)[o].astype(np.float32),
                                (c - ch * CH_ROWS)[o].astype(np.int16),
                                v[o]))

    cmax = counts.max(axis=0)
    G = np.maximum(1, np.ceil(cmax / TILE).astype(np.int64))
    slots = TILE * G
    offs = np.concatenate([[0], np.cumsum(slots)])
    S = int(offs[-1])

    per_core = []
    for p in range(N_CORES):
        b_s, dst_s, idx_s, val_s = per_core_sorted[p]
        cnt = counts[p]
        starts = np.concatenate([[0], np.cumsum(cnt)])
        pos = offs[b_s] + (np.arange(len(b_s)) - starts[b_s])
        idx = np.zeros(S, np.int16)
        dst = np.zeros(S, np.float32)
        val = np.zeros(S, np.float32)
        idx[pos] = idx_s
        dst[pos] = dst_s
        val[pos] = val_s
        per_core.append({
            "eidx": np.tile(idx.reshape(S // 16, 16).T, (8, 1)),   # [128, S/16]
            "edst": dst.reshape(S // TILE, TILE).T.copy(),          # [128, S/128]
            "eval": val.reshape(S // TILE, TILE).T.copy(),          # [128, S/128]
        })
    return G, offs, S, per_core


def _plan_pieces(G, offs):
    """Group consecutive buckets into idx-streaming pieces <= PIECE_SLOTS."""
    pieces = []  # (first_bucket, end_bucket, slot_off, n_slots)
    b = 0
    while b < NB:
        b0 = b
        s0 = int(offs[b])
        while b < NB and int(offs[b + 1]) - s0 <= PIECE_SLOTS:
            b += 1
        assert b > b0, "bucket larger than PIECE_SLOTS"
        pieces.append((b0, b, s0, int(offs[b]) - s0))
    return pieces


def build_program(G, offs, S, with_collectives=True):
    """Build the SPMD Bass program. Returns nc."""
    G = [int(g) for g in G]
    offs = [int(o) for o in offs]
    pieces = _plan_pieces(np.asarray(G), np.asarray(offs))

    nc = bacc.Bacc("TRN2", target_bir_lowering=False, debug=False,
                   num_devices=N_CORES, num_swdge_queues=N_QUEUES)

    xT_d = nc.declare_dram_parameter("xT", [IN_DIM, SLAB], BF16, isOutput=False)
    W1_d = nc.declare_dram_parameter("W1", [IN_DIM, HID], BF16, isOutput=False)
    W2_d = nc.declare_dram_parameter("W2", [HID, HID], BF16, isOutput=False)
    fc1W_d = nc.declare_dram_parameter("fc1W", [HID, 32], BF16, isOutput=False)
    fc2Wd_d = nc.declare_dram_parameter("fc2Wd", [32, 1], BF16, isOutput=False)
    bmat_d = nc.declare_dram_parameter("bmat", [TILE, 2 * HID], BF16, isOutput=False)
    onesm_d = nc.declare_dram_parameter("onesm", [TILE, TILE], BF16, isOutput=False)
    fc1b_d = nc.declare_dram_parameter("fc1b", [32, 1], F32, isOutput=False)
    fc2bd_d = nc.declare_dram_parameter("fc2bd", [1, 1], F32, isOutput=False)
    iota_d = nc.declare_dram_parameter("iota", [TILE, TILE], BF16, isOutput=False)
    eidx_d = nc.declare_dram_parameter("eidx", [TILE, S // 16], I16, isOutput=False)
    edst_d = nc.declare_dram_parameter("edst", [TILE, S // TILE], F32, isOutput=False)
    eval_d = nc.declare_dram_parameter("eval", [TILE, S // TILE], F32, isOutput=False)
    out_d = nc.declare_dram_parameter("out", [2, SLAB], F32, isOutput=True)

    ag_space = {"addr_space": "Shared"} if with_collectives else {}
    ag1_in = nc.dram_tensor("ag1_in", [SLAB, HID], BF16)
    ag1_out = nc.dram_tensor("ag1_out", [N_NODES, HID], BF16, **ag_space)
    ag2_in = nc.dram_tensor("ag2_in", [SLAB, HID], BF16)
    ag2_out = nc.dram_tensor("ag2_out", [N_NODES, HID], BF16, **ag_space)

    GMAX = max(G)
    rep = [list(range(N_CORES))]
    ACT = mybir.ActivationFunctionType

    with tile.TileContext(nc) as tc:
        with (
            tc.tile_pool(name="const", bufs=1) as constp,
            tc.tile_pool(name="big", bufs=1) as bigp,
            tc.tile_pool(name="edge", bufs=1) as edgep,
            tc.tile_pool(name="eidxp", bufs=2) as eidxp,
            tc.tile_pool(name="gath", bufs=GATH_BUFS) as gathp,
            tc.tile_pool(name="small", bufs=3) as smallp,
            tc.tile_pool(name="stage", bufs=2) as stagep,
            tc.tile_pool(name="ps_seg", bufs=2, space="PSUM") as ps_seg,
            tc.tile_pool(name="ps_mm", bufs=2, space="PSUM") as ps_mm,
            tc.tile_pool(name="ps_tr", bufs=2, space="PSUM") as ps_tr,
            tc.tile_pool(name="ps_hd", bufs=1, space="PSUM") as ps_hd,
        ):
            # ---- constants ----
            iota_t = constp.tile([TILE, TILE], BF16)
            ident = constp.tile([TILE, TILE], BF16)
            W1_t = constp.tile([TILE, 4, HID], BF16)
            W2_t = constp.tile([TILE, 2, HID], BF16)
            fc1W_t = constp.tile([TILE, 2, 32], BF16)
            fc2Wd_t = constp.tile([32, 1], BF16)
            bmat_t = constp.tile([TILE, 2 * HID], BF16)
            onesm_t = constp.tile([TILE, TILE], BF16)
            fc1b_t = constp.tile([32, 1], F32)
            fc2bd_t = constp.tile([1, 1], F32)
            edst_t = edgep.tile([TILE, S // TILE], F32)
            eval_t = edgep.tile([TILE, S // TILE], F32)

            nc.sync.dma_start(out=iota_t[:], in_=iota_d[:])
            make_identity(nc, ident[:])
            for k in range(4):
                nc.sync.dma_start(out=W1_t[:, k, :], in_=W1_d[k * TILE:(k + 1) * TILE, :])
            for k in range(2):
                nc.sync.dma_start(out=W2_t[:, k, :], in_=W2_d[k * TILE:(k + 1) * TILE, :])
                nc.sync.dma_start(out=fc1W_t[:, k, :], in_=fc1W_d[k * TILE:(k + 1) * TILE, :])
            nc.sync.dma_start(out=fc2Wd_t[:], in_=fc2Wd_d[:])
            nc.sync.dma_start(out=bmat_t[:], in_=bmat_d[:])
            nc.sync.dma_start(out=onesm_t[:], in_=onesm_d[:])
            nc.sync.dma_start(out=fc1b_t[:], in_=fc1b_d[:])
            nc.sync.dma_start(out=fc2bd_t[:], in_=fc2bd_d[:])
            nc.sync.dma_start(out=edst_t[:], in_=edst_d[:])
            nc.sync.dma_start(out=eval_t[:], in_=eval_d[:])
            b1row = bmat_t[:, 0:HID]
            b2row = bmat_t[:, HID:2 * HID]

            # pre-zero the gather buffers: skipped (-1) slots must read
            # finite bf16 values, and virgin SBUF is not guaranteed finite.
            for _ in range(GATH_BUFS):
                zt = gathp.tile([TILE, GMAX, HID], BF16, tag="gath")
                nc.vector.memset(zt[:], 0.0)

            def mm_slab(src_big, W_tile, nk, ag_in):
                """support = srcT @ W per dest tile, cast bf16, DMA to ag_in."""
                for t in range(NT):
                    rt = _rows_of_tile(t)
                    ps = ps_mm.tile([TILE, HID], F32, tag="mm")
                    for k in range(nk):
                        nc.tensor.matmul(
                            out=ps[:rt, :],
                            lhsT=src_big[:, k * SLAB + t * TILE:
                                         k * SLAB + t * TILE + rt],
                            rhs=W_tile[:, k, :],
                            start=(k == 0), stop=(k == nk - 1),
                        )
                    ev = smallp.tile([TILE, HID], BF16, tag="mmev")
                    nc.scalar.activation(ev[:rt, :], ps[:rt, :], ACT.Copy)
                    nc.sync.dma_start(out=ag_in[t * TILE:t * TILE + rt, :], in_=ev[:rt, :])

            def segsum(table, brow_b, evac):
                """SpMM via gather + one-hot matmul. evac(t, psum_ap)."""
                qn = 0
                for (b0, b1_, s0, nsl) in pieces:
                    ip = eidxp.tile([TILE, PIECE_SLOTS // 16], I16, tag="eidx")
                    nc.sync.dma_start(out=ip[:, :nsl // 16],
                                      in_=eidx_d[:, s0 // 16:(s0 + nsl) // 16])
                    for b in range(b0, b1_):
                        t, ch = b // N_CH, b % N_CH
                        rt = _rows_of_tile(t)
                        g0 = offs[b] // TILE
                        ng = G[b]
                        n_idx = ng * TILE
                        gt = gathp.tile([TILE, GMAX, HID], BF16, tag="gath")
                        nc.gpsimd.dma_gather(
                            gt[:, :ng, :],
                            table[ch * CH_ROWS:(ch + 1) * CH_ROWS, :],
                            ip[:, (offs[b] - s0) // 16:(offs[b] - s0 + n_idx) // 16],
                            n_idx, n_idx, HID,
                            single_packet=False,
                            queue_num=qn,
                        )
                        qn = (qn + 1) % N_QUEUES
                        if ch == 0:
                            ps = ps_seg.tile([TILE, HID], F32, tag="seg")
                            # bias fold: ps = onesm^T @ bmat (row 0 carries
                            # the bias; other partitions are zero)
                            nc.tensor.matmul(out=ps[:rt, :], lhsT=onesm_t[:, :rt],
                                             rhs=brow_b, start=True, stop=False)
                        for g in range(ng):
                            oh = smallp.tile([TILE, TILE], BF16, tag="oh")
                            nc.vector.tensor_scalar(
                                oh[:], iota_t[:],
                                edst_t[:, g0 + g:g0 + g + 1],
                                eval_t[:, g0 + g:g0 + g + 1],
                                mybir.AluOpType.is_equal, mybir.AluOpType.mult,
                            )
                            nc.tensor.matmul(
                                out=ps[:rt, :], lhsT=oh[:, :rt], rhs=gt[:, g, :],
                                start=False,
                                stop=(ch == N_CH - 1 and g == ng - 1),
                            )
                        if ch == N_CH - 1:
                            evac(t, ps)

            # ---- phase A: load xT (pre-cast bf16), mm1 ----
            xT = bigp.tile([TILE, 4 * SLAB], BF16, tag="big")
            for k in range(4):
                nc.sync.dma_start(out=xT[:, k * SLAB:(k + 1) * SLAB],
                                  in_=xT_d[k * TILE:(k + 1) * TILE, :])
            mm_slab(xT, W1_t, 4, ag1_in)

            if with_collectives:
                nc.gpsimd.collective_compute(
                    "AllGather", mybir.AluOpType.bypass,
                    ins=[ag1_in[:]], outs=[ag1_out[:]], replica_groups=rep)

            # ---- phase C: L1 SpMM -> h (node-major bf16) ----
            h = bigp.tile([TILE, NT * HID], BF16, tag="big")

            def evac1(t, ps):
                rt = _rows_of_tile(t)
                nc.scalar.activation(h[:rt, t * HID:t * HID + HID], ps[:rt, :],
                                     ACT.Relu)

            segsum(ag1_out, b1row, evac1)

            # ---- phase D: transpose h, mm2 ----
            for t in range(NT):
                rt = _rows_of_tile(t)
                hT = smallp.tile([TILE, 2, TILE], BF16, tag="hT")
                for k in range(2):
                    tp = ps_tr.tile([TILE, TILE], BF16, tag="tr")
                    nc.tensor.transpose(
                        out=tp[:, :rt],
                        in_=h[:rt, t * HID + k * TILE:t * HID + (k + 1) * TILE],
                        identity=ident[:rt, :rt])
                    nc.scalar.activation(hT[:, k, :rt], tp[:, :rt], ACT.Copy)
                ps = ps_mm.tile([TILE, HID], F32, tag="mm")
                for k in range(2):
                    nc.tensor.matmul(out=ps[:rt, :], lhsT=hT[:, k, :rt],
                                     rhs=W2_t[:, k, :],
                                     start=(k == 0), stop=(k == 1))
                ev = smallp.tile([TILE, HID], BF16, tag="mmev")
                nc.scalar.activation(ev[:rt, :], ps[:rt, :], ACT.Copy)
                nc.sync.dma_start(out=ag2_in[t * TILE:t * TILE + rt, :], in_=ev[:rt, :])

            if with_collectives:
                nc.gpsimd.collective_compute(
                    "AllGather", mybir.AluOpType.bypass,
                    ins=[ag2_in[:]], outs=[ag2_out[:]], replica_groups=rep)

            # ---- phase E: L2 SpMM -> logitsT (feature-major bf16) ----
            lgT = bigp.tile([TILE, 2 * SLAB], BF16, tag="big")

            def evac2(t, ps):
                rt = _rows_of_tile(t)
                ev = smallp.tile([TILE, HID], BF16, tag="ev")
                nc.scalar.activation(ev[:rt, :], ps[:rt, :], ACT.Copy)
                for k in range(2):
                    tp = ps_tr.tile([TILE, TILE], BF16, tag="tr")
                    nc.tensor.transpose(
                        out=tp[:, :rt],
                        in_=ev[:rt, k * TILE:(k + 1) * TILE],
                        identity=ident[:rt, :rt])
                    nc.scalar.activation(
                        lgT[:, k * SLAB + t * TILE:k * SLAB + t * TILE + rt],
                        tp[:, :rt], ACT.Copy)

            segsum(ag2_out, b2row, evac2)

            # ---- phase F: head + log_softmax (batched strips) ----
            for t0 in range(0, NT, HEAD_TILES):
                t1 = min(t0 + HEAD_TILES, NT)
                w = (t1 * TILE if t1 < NT else SLAB) - t0 * TILE
                dv = stagep.tile([1, HEAD_TILES * TILE], F32, tag="dv")
                for t in range(t0, t1):
                    rt = _rows_of_tile(t)
                    hp = ps_hd.tile([32, TILE], F32, tag="h1")
                    for k in range(2):
                        nc.tensor.matmul(out=hp[:, :rt], lhsT=fc1W_t[:, k, :],
                                         rhs=lgT[:, k * SLAB + t * TILE:k * SLAB + t * TILE + rt],
                                         start=(k == 0), stop=(k == 1))
                    haT = smallp.tile([32, TILE], BF16, tag="haT")
                    nc.scalar.activation(haT[:, :rt], hp[:, :rt], ACT.Relu,
                                         bias=fc1b_t[:])
                    zp = ps_hd.tile([1, TILE], F32, tag="h2")
                    nc.tensor.matmul(out=zp[:, :rt], lhsT=fc2Wd_t[:],
                                     rhs=haT[:, :rt], start=True, stop=True)
                    co = (t - t0) * TILE
                    nc.scalar.activation(dv[:, co:co + rt], zp[:, :rt],
                                         ACT.Identity, bias=fc2bd_t[:])
                # log_softmax over 2 classes (stable; Abs/Exp/Ln/Relu share
                # one ACT table): a = ln(1+exp(-|d|));
                # out0 = -relu(d) - a; out1 = out0 + d.
                ab = stagep.tile([1, HEAD_TILES * TILE], F32, tag="ab", bufs=1)
                nc.scalar.activation(ab[:, :w], dv[:, :w], ACT.Abs)
                ex = stagep.tile([1, HEAD_TILES * TILE], F32, tag="ex", bufs=1)
                nc.scalar.activation(ex[:, :w], ab[:, :w], ACT.Exp, scale=-1.0)
                nc.scalar.activation(ab[:, :w], ex[:, :w], ACT.Ln, bias=1.0)
                rl = stagep.tile([1, HEAD_TILES * TILE], F32, tag="rl", bufs=1)
                nc.scalar.activation(rl[:, :w], dv[:, :w], ACT.Relu)
                zo0 = stagep.tile([1, HEAD_TILES * TILE], F32, tag="zo0")
                nc.vector.scalar_tensor_tensor(
                    out=zo0[:, :w], in0=rl[:, :w], scalar=-1.0, in1=ab[:, :w],
                    op0=mybir.AluOpType.mult, op1=mybir.AluOpType.subtract)
                zo1 = stagep.tile([1, HEAD_TILES * TILE], F32, tag="zo1")
                nc.vector.tensor_tensor(
                    out=zo1[:, :w], in0=zo0[:, :w], in1=dv[:, :w],
                    op=mybir.AluOpType.add)
                nc.sync.dma_start(out=out_d[0:1, t0 * TILE:t0 * TILE + w],
                                  in_=zo0[:, :w])
                nc.sync.dma_start(out=out_d[1:2, t0 * TILE:t0 * TILE + w],
                                  in_=zo1[:, :w])

    nc.compile()
    return nc


def make_in_maps(inputs, per_core):
    """Build per-core input maps from full problem inputs."""
    x = np.asarray(inputs["inputs"], np.float32)
    W1 = np.ascontiguousarray(np.asarray(inputs["W1"], np.float32).astype(BF))
    W2 = np.ascontiguousarray(np.asarray(inputs["W2"], np.float32).astype(BF))
    fc1W = np.ascontiguousarray(np.asarray(inputs["fc1_W"], np.float32).astype(BF))
    fc2W = np.asarray(inputs["fc2_W"], np.float32)
    fc2Wd = np.ascontiguousarray((fc2W[:, 1] - fc2W[:, 0]).reshape(32, 1)).astype(BF)
    bmat = np.zeros((TILE, 2 * HID), np.float32)
    bmat[0, :HID] = np.asarray(inputs["b1"], np.float32)
    bmat[0, HID:] = np.asarray(inputs["b2"], np.float32)
    bmat = bmat.astype(BF)
    onesm = np.zeros((TILE, TILE), np.float32)
    onesm[0, :] = 1.0
    onesm = onesm.astype(BF)
    fc1b = np.asarray(inputs["fc1_b"], np.float32).reshape(32, 1)
    fc2b = np.asarray(inputs["fc2_b"], np.float32)
    fc2bd = np.array([[fc2b[1] - fc2b[0]]], np.float32)
    iota = np.tile(np.arange(TILE, dtype=np.float32), (TILE, 1)).astype(BF)

    in_maps = []
    for p in range(N_CORES):
        xT = np.ascontiguousarray(x[p * SLAB:(p + 1) * SLAB, :].T.astype(BF))
        m = {
            "xT": xT, "W1": W1, "W2": W2, "fc1W": fc1W, "fc2Wd": fc2Wd,
            "bmat": bmat, "onesm": onesm, "fc1b": fc1b, "fc2bd": fc2bd,
            "iota": iota,
            "eidx": per_core[p]["eidx"],
            "edst": per_core[p]["edst"],
            "eval": per_core[p]["eval"],
        }
        in_maps.append(m)
    return in_maps


LAST_RESULT = None


def kernel(**inputs):
    global LAST_RESULT
    G, offs, S, per_core = preprocess_edges(
        inputs["edge_rows"], inputs["edge_cols"], inputs["edge_vals"])
    nc = build_program(G, offs, S, with_collectives=True)
    in_maps = make_in_maps(inputs, per_core)
    res = run_bass_kernel_spmd(nc, in_maps, list(range(N_CORES)))
    LAST_RESULT = res
    out = np.concatenate([res.results[p]["out"].T for p in range(N_CORES)], axis=0)
    return np.ascontiguousarray(out.astype(np.float32))



# revision 5
# speedup vs baseline: 1.6094x; 1.6094x over previous
"""Trainium2 Bass kernel for a 2-layer GCN + MLP head (nn_GCN2).

Strategy (8 NeuronCores, SPMD):
  - Nodes sharded into 8 destination slabs of 12500; weights replicated.
  - Per GCN layer: each core computes its slab of support = x @ W on the
    TensorEngine (bf16), an AllGather builds the full [100000, 256] bf16
    support table in DRAM, then the SpMM (gather - scale - scatter-add)
    runs as per-edge dma_gather (512B rows) + one-hot matmuls on the
    TensorEngine accumulating into PSUM per 128-destination tile.
  - KEY throughput lever: dma_gather descriptor generation runs on the
    Q7 core-pair selected by queue_num. Calls on DIFFERENT queues
    overlap (~2.7ns/slot at 4-queue round-robin vs ~8ns serial), so the
    SpMM issues gathers round-robin over all 4 queues with 8 rotating
    buffers and keeps consumers off the Pool queue's critical path.
    Calls are merged across tile PAIRS (same chunk) to amortize the
    ~2.5us fixed per-call cost.
  - One-hot tiles are built BATCHED on the DVE (16 groups per op) from
    broadcast [128, 16, 1] -> [128, 16, 128] operands: is_equal against
    a repeated iota then multiply by edge vals. This replaces per-group
    tensor_scalar ops (765ns engine time each) that stalled the Pool
    queue via gather-buffer backpressure in the old design.
  - Do NOT pad gather idx slots with -1: the ucode's trailing-negative
    strip path hangs the device even when leading idxs are valid. Pads
    use idx 0 with val 0 (one-hot column is zero, so the contribution
    vanishes; slot data is finite).
  - Biases fold into the PSUM accumulation as a K=128 matmul (ones in
    partition 0 of lhsT x bias row in partition 0 of rhs), so PSUM
    evacuation is a single ACT activation (Relu / Copy).
  - The MLP head chains transposed (feature-on-partition) so fc1_W/fc2_W
    serve as lhsT directly; log_softmax over 2 classes runs in batched
    strips using only ACT table-6 functions: a = ln(1+exp(-|d|)),
    out0 = -relu(d)-a, out1 = out0 + d.
"""
import math
import sys

import numpy as np

for _p in ("/opt/trn_rl_repo",):
    if _p not in sys.path:
        sys.path.insert(0, _p)

import ml_dtypes  # noqa: E402
from concourse import bacc, bass, mybir  # noqa: E402
import concourse.tile as tile  # noqa: E402
from concourse.masks import make_identity  # noqa: E402
from concourse.bass_utils import run_bass_kernel_spmd  # noqa: E402

BF16 = mybir.dt.bfloat16
F32 = mybir.dt.float32
I16 = mybir.dt.int16
I32 = mybir.dt.int32
BF = ml_dtypes.bfloat16

N_CORES = 8
N_NODES = 100000
SLAB = N_NODES // N_CORES          # 12500 dest nodes per core
TILE = 128
NT = math.ceil(SLAB / TILE)        # 98 dest tiles (last has 84 rows)
NPAIR = NT // 2                    # 49 tile pairs
IN_DIM = 512
HID = 256
CH_ROWS = 25000                    # gather-table chunk (int16 index limit)
N_CH = N_NODES // CH_ROWS          # 4
NCALL = NPAIR * N_CH               # 196 gather calls per layer
PIECE_SLOTS = 24576                # idx streaming piece budget
N_QUEUES = 4                       # SWDGE queues (Q7 core pairs)
GATH_BUFS = 6
OH_BATCH = 16                      # groups per batched one-hot build
HEAD_TILES = 8                     # head tiles per strip (1024 cols)


def _rows_of_tile(t):
    return min(TILE, SLAB - t * TILE)


def preprocess_edges(edge_rows, edge_cols, edge_vals):
    """Sort/bucket edges per core; pad buckets uniformly across cores.

    Bucket order: (pair, ch, tile-in-pair) so one gather call covers the
    two buckets (2p, ch), (2p+1, ch) contiguously.

    Returns (G, offs, S, per_core): G[b]=groups of 128 slots in bucket
    b (order above), offs[b]=slot offset, S=total slots.
    """
    rows = edge_rows.astype(np.int64)
    cols = edge_cols.astype(np.int64)
    vals = np.asarray(edge_vals, np.float32)
    core = rows // SLAB

    NB = NT * N_CH

    def bucket_of(t, ch):
        return (t // 2) * (2 * N_CH) + ch * 2 + (t % 2)

    per_core_sorted = []
    counts = np.zeros((N_CORES, NB), np.int64)
    tt = np.zeros(NB, np.int64)  # bucket -> tile
    for t in range(NT):
        for ch in range(N_CH):
            tt[bucket_of(t, ch)] = t
    for p in range(N_CORES):
        sel = np.nonzero(core == p)[0]
        r = rows[sel] - p * SLAB
        c = cols[sel]
        v = vals[sel]
        t = r // TILE
        ch = c // CH_ROWS
        b = (t // 2) * (2 * N_CH) + ch * 2 + (t % 2)
        o = np.argsort(b, kind="stable")
        b_s = b[o]
        counts[p] = np.bincount(b_s, minlength=NB)
        per_core_sorted.append((b_s, (r % TILE)[o].astype(np.float32),
                                (c - ch * CH_ROWS)[o].astype(np.int16),
                                v[o]))

    cmax = counts.max(axis=0)
    G = np.maximum(1, np.ceil(cmax / TILE).astype(np.int64))
    slots = TILE * G
    offs = np.concatenate([[0], np.cumsum(slots)])
    S = int(offs[-1])

    per_core = []
    for p in range(N_CORES):
        b_s, dst_s, idx_s, val_s = per_core_sorted[p]
        cnt = counts[p]
        starts = np.concatenate([[0], np.cumsum(cnt)])
        pos = offs[b_s] + (np.arange(len(b_s)) - starts[b_s])
        idx = np.zeros(S, np.int16)
        dst = np.zeros(S, np.float32)
        val = np.zeros(S, np.float32)
        idx[pos] = idx_s
        dst[pos] = dst_s
        val[pos] = val_s
        per_core.append({
            "eidx": np.tile(idx.reshape(S // 16, 16).T, (8, 1)),   # [128, S/16]
            "edst": dst.reshape(S // TILE, TILE).T.copy(),          # [128, NG]
            "eval": val.reshape(S // TILE, TILE).T.copy(),          # [128, NG]
        })
    return G, offs, S, per_core


def _plan_pieces(G, offs):
    """Group consecutive gather CALLS into idx pieces <= PIECE_SLOTS.

    A call c = pair*N_CH + ch covers buckets 2c, 2c+1 (in bucket order).
    Returns list of (call0, call1, slot_off, n_slots).
    """
    pieces = []
    c = 0
    while c < NCALL:
        c0 = c
        s0 = int(offs[2 * c])
        while c < NCALL and int(offs[2 * c + 2]) - s0 <= PIECE_SLOTS:
            c += 1
        assert c > c0, "call larger than PIECE_SLOTS"
        pieces.append((c0, c, s0, int(offs[2 * c]) - s0))
    return pieces


def build_program(G, offs, S, with_collectives=True):
    """Build the SPMD Bass program. Returns nc."""
    G = [int(g) for g in G]
    offs = [int(o) for o in offs]
    pieces = _plan_pieces(np.asarray(G), np.asarray(offs))
    NG = S // TILE                    # total one-hot groups per layer
    NGB = math.ceil(NG / OH_BATCH)    # one-hot build batches
    NGP = NGB * OH_BATCH

    nc = bacc.Bacc("TRN2", target_bir_lowering=False, debug=False,
                   num_devices=N_CORES, num_swdge_queues=N_QUEUES)

    xT_d = nc.declare_dram_parameter("xT", [IN_DIM, SLAB], BF16, isOutput=False)
    W1_d = nc.declare_dram_parameter("W1", [IN_DIM, HID], BF16, isOutput=False)
    W2_d = nc.declare_dram_parameter("W2", [HID, HID], BF16, isOutput=False)
    fc1W_d = nc.declare_dram_parameter("fc1W", [HID, 32], BF16, isOutput=False)
    fc2Wd_d = nc.declare_dram_parameter("fc2Wd", [32, 1], BF16, isOutput=False)
    bmat_d = nc.declare_dram_parameter("bmat", [TILE, 2 * HID], BF16, isOutput=False)
    onesm_d = nc.declare_dram_parameter("onesm", [TILE, TILE], BF16, isOutput=False)
    fc1b_d = nc.declare_dram_parameter("fc1b", [32, 1], F32, isOutput=False)
    fc2bd_d = nc.declare_dram_parameter("fc2bd", [1, 1], F32, isOutput=False)
    iota_d = nc.declare_dram_parameter("iota", [TILE, OH_BATCH, TILE], BF16,
                                       isOutput=False)
    eidx_d = nc.declare_dram_parameter("eidx", [TILE, S // 16], I16, isOutput=False)
    edst_d = nc.declare_dram_parameter("edst", [TILE, NGP], F32, isOutput=False)
    eval_d = nc.declare_dram_parameter("eval", [TILE, NGP], F32, isOutput=False)
    out_d = nc.declare_dram_parameter("out", [2, SLAB], F32, isOutput=True)

    ag_space = {"addr_space": "Shared"} if with_collectives else {}
    ag1_in = nc.dram_tensor("ag1_in", [SLAB, HID], BF16)
    ag1_out = nc.dram_tensor("ag1_out", [N_NODES, HID], BF16, **ag_space)
    ag2_in = nc.dram_tensor("ag2_in", [SLAB, HID], BF16)
    ag2_out = nc.dram_tensor("ag2_out", [N_NODES, HID], BF16, **ag_space)

    # max slots in one merged gather call
    CMAX = max(G[2 * c] + G[2 * c + 1] for c in range(NCALL))
    rep = [list(range(N_CORES))]
    ACT = mybir.ActivationFunctionType

    with tile.TileContext(nc) as tc:
        with (
            tc.tile_pool(name="const", bufs=1) as constp,
            tc.tile_pool(name="big", bufs=1) as bigp,
            tc.tile_pool(name="edge", bufs=1) as edgep,
            tc.tile_pool(name="eidxp", bufs=2) as eidxp,
            tc.tile_pool(name="gath", bufs=GATH_BUFS) as gathp,
            tc.tile_pool(name="ohp", bufs=4) as ohp,
            tc.tile_pool(name="xs", bufs=3) as xsp,
            tc.tile_pool(name="small", bufs=3) as smallp,
            tc.tile_pool(name="stage", bufs=2) as stagep,
            tc.tile_pool(name="ps_seg", bufs=2, space="PSUM") as ps_seg,
            tc.tile_pool(name="ps_mm", bufs=2, space="PSUM") as ps_mm,
            tc.tile_pool(name="ps_tr", bufs=2, space="PSUM") as ps_tr,
            tc.tile_pool(name="ps_hd", bufs=1, space="PSUM") as ps_hd,
        ):
            # ---- constants ----
            iota_t = constp.tile([TILE, OH_BATCH, TILE], BF16)
            ident = constp.tile([TILE, TILE], BF16)
            W1_t = constp.tile([TILE, 4, HID], BF16)
            W2_t = constp.tile([TILE, 2, HID], BF16)
            fc1W_t = constp.tile([TILE, 2, 32], BF16)
            fc2Wd_t = constp.tile([32, 1], BF16)
            bmat_t = constp.tile([TILE, 2 * HID], BF16)
            onesm_t = constp.tile([TILE, TILE], BF16)
            fc1b_t = constp.tile([32, 1], F32)
            fc2bd_t = constp.tile([1, 1], F32)
            edst_t = edgep.tile([TILE, NGP, 1], F32)
            eval_t = edgep.tile([TILE, NGP, 1], F32)

            nc.sync.dma_start(out=iota_t[:], in_=iota_d[:])
            make_identity(nc, ident[:])
            for k in range(4):
                nc.sync.dma_start(out=W1_t[:, k, :], in_=W1_d[k * TILE:(k + 1) * TILE, :])
            for k in range(2):
                nc.sync.dma_start(out=W2_t[:, k, :], in_=W2_d[k * TILE:(k + 1) * TILE, :])
                nc.sync.dma_start(out=fc1W_t[:, k, :], in_=fc1W_d[k * TILE:(k + 1) * TILE, :])
            nc.sync.dma_start(out=fc2Wd_t[:], in_=fc2Wd_d[:])
            nc.sync.dma_start(out=bmat_t[:], in_=bmat_d[:])
            nc.sync.dma_start(out=onesm_t[:], in_=onesm_d[:])
            nc.sync.dma_start(out=fc1b_t[:], in_=fc1b_d[:])
            nc.sync.dma_start(out=fc2bd_t[:], in_=fc2bd_d[:])
            nc.sync.dma_start(out=edst_t[:, :, 0], in_=edst_d[:])
            nc.sync.dma_start(out=eval_t[:, :, 0], in_=eval_d[:])
            b1row = bmat_t[:, 0:HID]
            b2row = bmat_t[:, HID:2 * HID]

            # pre-zero the gather buffers: pad slots (idx 0, val 0) write
            # real rows, but short-stripped groups never write, and virgin
            # SBUF is not guaranteed finite bf16.
            for _ in range(GATH_BUFS):
                zt = gathp.tile([TILE, CMAX, HID], BF16, tag="gath")
                nc.vector.memset(zt[:], 0.0)

            def mm_slab(W_tile, nk, src_dram, src_cols, ag_in):
                """support = srcT @ W per dest tile, cast bf16, DMA to ag_in.

                src_dram: [nk*TILE, src_cols] bf16 (feature-major), streamed
                per tile so the matmul starts without a bulk preload.
                """
                for t in range(NT):
                    rt = _rows_of_tile(t)
                    xs = xsp.tile([TILE, nk, TILE], BF16, tag="xs")
                    for k in range(nk):
                        nc.sync.dma_start(
                            out=xs[:, k, :rt],
                            in_=src_dram[k * TILE:(k + 1) * TILE,
                                         t * TILE:t * TILE + rt])
                    ps = ps_mm.tile([TILE, HID], F32, tag="mm")
                    for k in range(nk):
                        nc.tensor.matmul(
                            out=ps[:rt, :],
                            lhsT=xs[:, k, :rt],
                            rhs=W_tile[:, k, :],
                            start=(k == 0), stop=(k == nk - 1),
                        )
                    ev = smallp.tile([TILE, HID], BF16, tag="mmev")
                    nc.scalar.activation(ev[:rt, :], ps[:rt, :], ACT.Copy)
                    nc.sync.dma_start(out=ag_in[t * TILE:t * TILE + rt, :],
                                      in_=ev[:rt, :])

            def segsum(table, brow_b, evac):
                """SpMM via queue-parallel gathers + one-hot matmuls.

                evac(t, psum_ap) consumes each finished dest tile.
                One gather call covers tile pair (2p, 2p+1) x chunk ch.
                One-hot tiles are built in batches of OH_BATCH groups on
                the DVE from broadcast compares against iota.
                """
                qn = 0
                oh_tiles = {}          # batch index -> tile
                next_batch = [0]

                def oh_of(g):
                    bi = g // OH_BATCH
                    while next_batch[0] <= bi:
                        k = next_batch[0]
                        ohb = ohp.tile([TILE, OH_BATCH, TILE], BF16, tag="oh")
                        nc.vector.tensor_tensor(
                            out=ohb[:],
                            in0=iota_t[:],
                            in1=edst_t[:, k * OH_BATCH:(k + 1) * OH_BATCH, :]
                                .to_broadcast([TILE, OH_BATCH, TILE]),
                            op=mybir.AluOpType.is_equal)
                        nc.vector.tensor_tensor(
                            out=ohb[:],
                            in0=ohb[:],
                            in1=eval_t[:, k * OH_BATCH:(k + 1) * OH_BATCH, :]
                                .to_broadcast([TILE, OH_BATCH, TILE]),
                            op=mybir.AluOpType.mult)
                        oh_tiles[k] = ohb
                        if k - 4 in oh_tiles:
                            del oh_tiles[k - 4]
                        next_batch[0] += 1
                    return oh_tiles[bi][:, g % OH_BATCH, :]

                ps_of = {}
                for (c0, c1, s0, nsl) in pieces:
                    ip = eidxp.tile([TILE, PIECE_SLOTS // 16], I16, tag="eidx")
                    nc.sync.dma_start(out=ip[:, :nsl // 16],
                                      in_=eidx_d[:, s0 // 16:(s0 + nsl) // 16])
                    for c in range(c0, c1):
                        pair, ch = c // N_CH, c % N_CH
                        ga, gb = G[2 * c], G[2 * c + 1]
                        n_idx = (ga + gb) * TILE
                        co = offs[2 * c]
                        gt = gathp.tile([TILE, CMAX, HID], BF16, tag="gath")
                        nc.gpsimd.dma_gather(
                            gt[:, :ga + gb, :],
                            table[ch * CH_ROWS:(ch + 1) * CH_ROWS, :],
                            ip[:, (co - s0) // 16:(co - s0 + n_idx) // 16],
                            n_idx, n_idx, HID,
                            single_packet=False,
                            queue_num=qn,
                        )
                        qn = (qn + 1) % N_QUEUES
                        for half, gn in ((0, ga), (1, gb)):
                            t = 2 * pair + half
                            rt = _rows_of_tile(t)
                            if ch == 0:
                                ps = ps_seg.tile([TILE, HID], F32, tag="seg")
                                ps_of[t] = ps
                                # bias fold: row 0 of onesm^T is ones
                                nc.tensor.matmul(out=ps[:rt, :],
                                                 lhsT=onesm_t[:, :rt],
                                                 rhs=brow_b,
                                                 start=True, stop=False)
                            ps = ps_of[t]
                            g0 = offs[2 * c + half] // TILE
                            for g in range(gn):
                                nc.tensor.matmul(
                                    out=ps[:rt, :],
                                    lhsT=oh_of(g0 + g)[:, :rt],
                                    rhs=gt[:, (half * ga) + g, :],
                                    start=False,
                                    stop=(ch == N_CH - 1 and g == gn - 1),
                                )
                            if ch == N_CH - 1:
                                evac(t, ps)
                                del ps_of[t]

            # ---- phase A: mm1 (xT streamed per tile) ----
            mm_slab(W1_t, 4, xT_d, SLAB, ag1_in)

            if with_collectives:
                nc.gpsimd.collective_compute(
                    "AllGather", mybir.AluOpType.bypass,
                    ins=[ag1_in[:]], outs=[ag1_out[:]], replica_groups=rep)

            # ---- phase C: L1 SpMM -> h (node-major bf16) ----
            h = bigp.tile([TILE, NT * HID], BF16, tag="big")

            def evac1(t, ps):
                rt = _rows_of_tile(t)
                nc.scalar.activation(h[:rt, t * HID:t * HID + HID], ps[:rt, :],
                                     ACT.Relu)

            segsum(ag1_out, b1row, evac1)

            # ---- phase D: transpose h, mm2 ----
            for t in range(NT):
                rt = _rows_of_tile(t)
                hT = smallp.tile([TILE, 2, TILE], BF16, tag="hT")
                for k in range(2):
                    tp = ps_tr.tile([TILE, TILE], BF16, tag="tr")
                    nc.tensor.transpose(
                        out=tp[:, :rt],
                        in_=h[:rt, t * HID + k * TILE:t * HID + (k + 1) * TILE],
                        identity=ident[:rt, :rt])
                    nc.scalar.activation(hT[:, k, :rt], tp[:, :rt], ACT.Copy)
                ps = ps_mm.tile([TILE, HID], F32, tag="mm")
                for k in range(2):
                    nc.tensor.matmul(out=ps[:rt, :], lhsT=hT[:, k, :rt],
                                     rhs=W2_t[:, k, :],
                                     start=(k == 0), stop=(k == 1))
                ev = smallp.tile([TILE, HID], BF16, tag="mmev")
                nc.scalar.activation(ev[:rt, :], ps[:rt, :], ACT.Copy)
                nc.sync.dma_start(out=ag2_in[t * TILE:t * TILE + rt, :], in_=ev[:rt, :])

            if with_collectives:
                nc.gpsimd.collective_compute(
                    "AllGather", mybir.AluOpType.bypass,
                    ins=[ag2_in[:]], outs=[ag2_out[:]], replica_groups=rep)

            # ---- phase E: L2 SpMM -> logitsT (feature-major bf16) ----
            lgT = bigp.tile([TILE, 2 * SLAB], BF16, tag="big")

            def evac2(t, ps):
                rt = _rows_of_tile(t)
                ev = smallp.tile([TILE, HID], BF16, tag="ev")
                nc.scalar.activation(ev[:rt, :], ps[:rt, :], ACT.Copy)
                for k in range(2):
                    tp = ps_tr.tile([TILE, TILE], BF16, tag="tr")
                    nc.tensor.transpose(
                        out=tp[:, :rt],
                        in_=ev[:rt, k * TILE:(k + 1) * TILE],
                        identity=ident[:rt, :rt])
                    nc.scalar.activation(
                        lgT[:, k * SLAB + t * TILE:k * SLAB + t * TILE + rt],
                        tp[:, :rt], ACT.Copy)

            segsum(ag2_out, b2row, evac2)

            # ---- phase F: head + log_softmax (batched strips) ----
            for t0 in range(0, NT, HEAD_TILES):
                t1 = min(t0 + HEAD_TILES, NT)
                w = (t1 * TILE if t1 < NT else SLAB) - t0 * TILE
                dv = stagep.tile([1, HEAD_TILES * TILE], F32, tag="dv")
                for t in range(t0, t1):
                    rt = _rows_of_tile(t)
                    hp = ps_hd.tile([32, TILE], F32, tag="h1")
                    for k in range(2):
                        nc.tensor.matmul(out=hp[:, :rt], lhsT=fc1W_t[:, k, :],
                                         rhs=lgT[:, k * SLAB + t * TILE:k * SLAB + t * TILE + rt],
                                         start=(k == 0), stop=(k == 1))
                    haT = smallp.tile([32, TILE], BF16, tag="haT")
                    nc.scalar.activation(haT[:, :rt], hp[:, :rt], ACT.Relu,
                                         bias=fc1b_t[:])
                    zp = ps_hd.tile([1, TILE], F32, tag="h2")
                    nc.tensor.matmul(out=zp[:, :rt], lhsT=fc2Wd_t[:],
                                     rhs=haT[:, :rt], start=True, stop=True)
                    co = (t - t0) * TILE
                    nc.scalar.activation(dv[:, co:co + rt], zp[:, :rt],
                                         ACT.Identity, bias=fc2bd_t[:])
                # log_softmax over 2 classes (stable; Abs/Exp/Ln/Relu share
                # one ACT table): a = ln(1+exp(-|d|));
                # out0 = -relu(d) - a; out1 = out0 + d.
                ab = stagep.tile([1, HEAD_TILES * TILE], F32, tag="ab", bufs=1)
                nc.scalar.activation(ab[:, :w], dv[:, :w], ACT.Abs)
                ex = stagep.tile([1, HEAD_TILES * TILE], F32, tag="ex", bufs=1)
                nc.scalar.activation(ex[:, :w], ab[:, :w], ACT.Exp, scale=-1.0)
                nc.scalar.activation(ab[:, :w], ex[:, :w], ACT.Ln, bias=1.0)
                rl = stagep.tile([1, HEAD_TILES * TILE], F32, tag="rl", bufs=1)
                nc.scalar.activation(rl[:, :w], dv[:, :w], ACT.Relu)
                zo0 = stagep.tile([1, HEAD_TILES * TILE], F32, tag="zo0")
                nc.vector.scalar_tensor_tensor(
                    out=zo0[:, :w], in0=rl[:, :w], scalar=-1.0, in1=ab[:, :w],
                    op0=mybir.AluOpType.mult, op1=mybir.AluOpType.subtract)
                zo1 = stagep.tile([1, HEAD_TILES * TILE], F32, tag="zo1")
                nc.vector.tensor_tensor(
                    out=zo1[:, :w], in0=zo0[:, :w], in1=dv[:, :w],
                    op=mybir.AluOpType.add)
                nc.sync.dma_start(out=out_d[0:1, t0 * TILE:t0 * TILE + w],
                                  in_=zo0[:, :w])
                nc.sync.dma_start(out=out_d[1:2, t0 * TILE:t0 * TILE + w],
                                  in_=zo1[:, :w])

    nc.compile()
    return nc


def make_in_maps(inputs, per_core, S):
    """Build per-core input maps from full problem inputs."""
    NG = S // TILE
    NGP = math.ceil(NG / OH_BATCH) * OH_BATCH
    x = np.asarray(inputs["inputs"], np.float32)
    W1 = np.ascontiguousarray(np.asarray(inputs["W1"], np.float32).astype(BF))
    W2 = np.ascontiguousarray(np.asarray(inputs["W2"], np.float32).astype(BF))
    fc1W = np.ascontiguousarray(np.asarray(inputs["fc1_W"], np.float32).astype(BF))
    fc2W = np.asarray(inputs["fc2_W"], np.float32)
    fc2Wd = np.ascontiguousarray((fc2W[:, 1] - fc2W[:, 0]).reshape(32, 1)).astype(BF)
    bmat = np.zeros((TILE, 2 * HID), np.float32)
    bmat[0, :HID] = np.asarray(inputs["b1"], np.float32)
    bmat[0, HID:] = np.asarray(inputs["b2"], np.float32)
    bmat = bmat.astype(BF)
    onesm = np.zeros((TILE, TILE), np.float32)
    onesm[0, :] = 1.0
    onesm = onesm.astype(BF)
    fc1b = np.asarray(inputs["fc1_b"], np.float32).reshape(32, 1)
    fc2b = np.asarray(inputs["fc2_b"], np.float32)
    fc2bd = np.array([[fc2b[1] - fc2b[0]]], np.float32)
    iota = np.tile(np.arange(TILE, dtype=np.float32),
                   (TILE, OH_BATCH, 1)).astype(BF)

    in_maps = []
    for p in range(N_CORES):
        xT = np.ascontiguousarray(x[p * SLAB:(p + 1) * SLAB, :].T.astype(BF))
        edst = np.zeros((TILE, NGP), np.float32)
        edst[:, :NG] = per_core[p]["edst"]
        evalv = np.zeros((TILE, NGP), np.float32)
        evalv[:, :NG] = per_core[p]["eval"]
        m = {
            "xT": xT, "W1": W1, "W2": W2, "fc1W": fc1W, "fc2Wd": fc2Wd,
            "bmat": bmat, "onesm": onesm, "fc1b": fc1b, "fc2bd": fc2bd,
            "iota": iota,
            "eidx": per_core[p]["eidx"],
            "edst": np.ascontiguousarray(edst),
            "eval": np.ascontiguousarray(evalv),
        }
        in_maps.append(m)
    return in_maps


LAST_RESULT = None


def kernel(**inputs):
    global LAST_RESULT
    G, offs, S, per_core = preprocess_edges(
        inputs["edge_rows"], inputs["edge_cols"], inputs["edge_vals"])
    nc = build_program(G, offs, S, with_collectives=True)
    in_maps = make_in_maps(inputs, per_core, S)
    res = run_bass_kernel_spmd(nc, in_maps, list(range(N_CORES)))
    LAST_RESULT = res
    out = np.concatenate([res.results[p]["out"].T for p in range(N_CORES)], axis=0)
    return np.ascontiguousarray(out.astype(np.float32))


# revision 9
# speedup vs baseline: 1.7320x; 1.0762x over previous
"""Trainium2 Bass kernel for a 2-layer GCN + MLP head (nn_GCN2).

Strategy (8 NeuronCores, SPMD):
  - Nodes sharded into 8 destination slabs of 12500; weights replicated.
  - Per GCN layer: each core computes its slab of support = x @ W on the
    TensorEngine (bf16), an AllGather builds the full [100000, 256] bf16
    support table in DRAM, then the SpMM (gather - scale - scatter-add)
    runs as per-edge dma_gather (512B rows) + one-hot matmuls on the
    TensorEngine accumulating into PSUM per 128-destination tile.
  - KEY throughput lever: dma_gather descriptor generation runs on the
    Q7 core-pair selected by queue_num. Calls on DIFFERENT queues
    overlap (~2.7ns/slot at 4-queue round-robin vs ~8ns serial), so the
    SpMM issues gathers round-robin over all 4 queues with 8 rotating
    buffers and keeps consumers off the Pool queue's critical path.
    Calls are merged across tile PAIRS (same chunk) to amortize the
    ~2.5us fixed per-call cost.
  - One-hot tiles are built BATCHED on the DVE (16 groups per op) from
    broadcast [128, 16, 1] -> [128, 16, 128] operands: is_equal against
    a repeated iota then multiply by edge vals. This replaces per-group
    tensor_scalar ops (765ns engine time each) that stalled the Pool
    queue via gather-buffer backpressure in the old design.
  - Do NOT pad gather idx slots with -1: the ucode's trailing-negative
    strip path hangs the device even when leading idxs are valid. Pads
    use idx 0 with val 0 (one-hot column is zero, so the contribution
    vanishes; slot data is finite).
  - Biases fold into the PSUM accumulation as a K=128 matmul (ones in
    partition 0 of lhsT x bias row in partition 0 of rhs), so PSUM
    evacuation is a single ACT activation (Relu / Copy).
  - The MLP head chains transposed (feature-on-partition) so fc1_W/fc2_W
    serve as lhsT directly; log_softmax over 2 classes runs in batched
    strips using only ACT table-6 functions: a = ln(1+exp(-|d|)),
    out0 = -relu(d)-a, out1 = out0 + d.
"""
import math
import sys

import numpy as np

for _p in ("/opt/trn_rl_repo",):
    if _p not in sys.path:
        sys.path.insert(0, _p)

import ml_dtypes  # noqa: E402
from concourse import bacc, bass, mybir  # noqa: E402
import concourse.tile as tile  # noqa: E402
from concourse.masks import make_identity  # noqa: E402
from concourse.bass_utils import run_bass_kernel_spmd  # noqa: E402

BF16 = mybir.dt.bfloat16
F32 = mybir.dt.float32
I16 = mybir.dt.int16
I32 = mybir.dt.int32
BF = ml_dtypes.bfloat16

N_CORES = 8
N_NODES = 100000
SLAB = N_NODES // N_CORES          # 12500 dest nodes per core
TILE = 128
NT = math.ceil(SLAB / TILE)        # 98 dest tiles (last has 84 rows)
NPAIR = NT // 2                    # 49 tile pairs
IN_DIM = 512
HID = 256
CH_ROWS = 25000                    # gather-table chunk (int16 index limit)
N_CH = N_NODES // CH_ROWS          # 4
NCALL = NPAIR * N_CH               # 196 gather calls per layer
PIECE_SLOTS = 24576                # idx streaming piece budget
N_QUEUES = 4                       # SWDGE queues (Q7 core pairs)
GATH_BUFS = 5
OH_BATCH = 16                      # groups per batched one-hot build
HEAD_TILES = 8                     # head tiles per strip (1024 cols)


def _rows_of_tile(t):
    return min(TILE, SLAB - t * TILE)


def preprocess_edges(edge_rows, edge_cols, edge_vals):
    """Sort/bucket edges per core; pad buckets uniformly across cores.

    Bucket order: (pair, ch, tile-in-pair) so one gather call covers the
    two buckets (2p, ch), (2p+1, ch) contiguously.

    Returns (G, offs, S, per_core): G[b]=groups of 128 slots in bucket
    b (order above), offs[b]=slot offset, S=total slots.
    """
    rows = edge_rows.astype(np.int64)
    cols = edge_cols.astype(np.int64)
    vals = np.asarray(edge_vals, np.float32)
    core = rows // SLAB

    NB = NT * N_CH

    def bucket_of(t, ch):
        return (t // 2) * (2 * N_CH) + ch * 2 + (t % 2)

    per_core_sorted = []
    counts = np.zeros((N_CORES, NB), np.int64)
    tt = np.zeros(NB, np.int64)  # bucket -> tile
    for t in range(NT):
        for ch in range(N_CH):
            tt[bucket_of(t, ch)] = t
    for p in range(N_CORES):
        sel = np.nonzero(core == p)[0]
        r = rows[sel] - p * SLAB
        c = cols[sel]
        v = vals[sel]
        t = r // TILE
        ch = c // CH_ROWS
        b = (t // 2) * (2 * N_CH) + ch * 2 + (t % 2)
        o = np.argsort(b, kind="stable")
        b_s = b[o]
        counts[p] = np.bincount(b_s, minlength=NB)
        per_core_sorted.append((b_s, (r % TILE)[o].astype(np.float32),
                                (c - ch * CH_ROWS)[o].astype(np.int16),
                                v[o]))

    cmax = counts.max(axis=0)
    G = np.maximum(1, np.ceil(cmax / TILE).astype(np.int64))
    slots = TILE * G
    offs = np.concatenate([[0], np.cumsum(slots)])
    S = int(offs[-1])

    per_core = []
    for p in range(N_CORES):
        b_s, dst_s, idx_s, val_s = per_core_sorted[p]
        cnt = counts[p]
        starts = np.concatenate([[0], np.cumsum(cnt)])
        pos = offs[b_s] + (np.arange(len(b_s)) - starts[b_s])
        idx = np.zeros(S, np.int16)
        dst = np.zeros(S, np.float32)
        val = np.zeros(S, np.float32)
        idx[pos] = idx_s
        dst[pos] = dst_s
        val[pos] = val_s
        per_core.append({
            "eidx": np.tile(idx.reshape(S // 16, 16).T, (8, 1)),   # [128, S/16]
            "edst": dst.reshape(S // TILE, TILE).T.copy(),          # [128, NG]
            "eval": val.reshape(S // TILE, TILE).T.copy(),          # [128, NG]
        })
    return G, offs, S, per_core


def _plan_pieces(G, offs):
    """Group consecutive gather CALLS into idx pieces <= PIECE_SLOTS.

    A call c = pair*N_CH + ch covers buckets 2c, 2c+1 (in bucket order).
    Returns list of (call0, call1, slot_off, n_slots).
    """
    pieces = []
    c = 0
    while c < NCALL:
        c0 = c
        s0 = int(offs[2 * c])
        while c < NCALL and int(offs[2 * c + 2]) - s0 <= PIECE_SLOTS:
            c += 1
        assert c > c0, "call larger than PIECE_SLOTS"
        pieces.append((c0, c, s0, int(offs[2 * c]) - s0))
    return pieces


def build_program(G, offs, S, with_collectives=True):
    """Build the SPMD Bass program. Returns nc."""
    G = [int(g) for g in G]
    offs = [int(o) for o in offs]
    pieces = _plan_pieces(np.asarray(G), np.asarray(offs))
    NG = S // TILE                    # total one-hot groups per layer
    NGB = math.ceil(NG / OH_BATCH)    # one-hot build batches
    NGP = NGB * OH_BATCH

    nc = bacc.Bacc("TRN2", target_bir_lowering=False, debug=False,
                   num_devices=N_CORES, num_swdge_queues=N_QUEUES)

    xT_d = nc.declare_dram_parameter("xT", [IN_DIM, SLAB], BF16, isOutput=False)
    W1_d = nc.declare_dram_parameter("W1", [IN_DIM, HID], BF16, isOutput=False)
    W2_d = nc.declare_dram_parameter("W2", [HID, HID], BF16, isOutput=False)
    fc1W_d = nc.declare_dram_parameter("fc1W", [HID, 32], BF16, isOutput=False)
    fc2Wd_d = nc.declare_dram_parameter("fc2Wd", [32, 1], BF16, isOutput=False)
    bmat_d = nc.declare_dram_parameter("bmat", [TILE, 2 * HID], BF16, isOutput=False)
    onesm_d = nc.declare_dram_parameter("onesm", [TILE, TILE], BF16, isOutput=False)
    fc1b_d = nc.declare_dram_parameter("fc1b", [32, 1], F32, isOutput=False)
    fc2bd_d = nc.declare_dram_parameter("fc2bd", [1, 1], F32, isOutput=False)
    iota_d = nc.declare_dram_parameter("iota", [TILE, OH_BATCH, TILE], BF16,
                                       isOutput=False)
    eidx_d = nc.declare_dram_parameter("eidx", [TILE, S // 16], I16, isOutput=False)
    edst_d = nc.declare_dram_parameter("edst", [TILE, NGP], BF16, isOutput=False)
    eval_d = nc.declare_dram_parameter("eval", [TILE, NGP], BF16, isOutput=False)
    out_d = nc.declare_dram_parameter("out", [2, SLAB], F32, isOutput=True)

    ag_space = {"addr_space": "Shared"} if with_collectives else {}
    ag1_in = nc.dram_tensor("ag1_in", [SLAB, HID], BF16)
    ag1_out = nc.dram_tensor("ag1_out", [N_NODES, HID], BF16, **ag_space)
    ag2_in = nc.dram_tensor("ag2_in", [SLAB, HID], BF16)
    ag2_out = nc.dram_tensor("ag2_out", [N_NODES, HID], BF16, **ag_space)

    # max slots in one merged gather call
    CMAX = max(G[2 * c] + G[2 * c + 1] for c in range(NCALL))
    rep = [list(range(N_CORES))]
    ACT = mybir.ActivationFunctionType

    with tile.TileContext(nc) as tc:
        with (
            tc.tile_pool(name="const", bufs=1) as constp,
            tc.tile_pool(name="big", bufs=1) as bigp,
            tc.tile_pool(name="edge", bufs=1) as edgep,
            tc.tile_pool(name="eidxp", bufs=2) as eidxp,
            tc.tile_pool(name="gath", bufs=GATH_BUFS) as gathp,
            tc.tile_pool(name="ohp", bufs=4) as ohp,
            tc.tile_pool(name="xs", bufs=2) as xsp,
            tc.tile_pool(name="small", bufs=3) as smallp,
            tc.tile_pool(name="stage", bufs=2) as stagep,
            tc.tile_pool(name="ps_seg", bufs=2, space="PSUM") as ps_seg,
            tc.tile_pool(name="ps_mm", bufs=2, space="PSUM") as ps_mm,
            tc.tile_pool(name="ps_tr", bufs=2, space="PSUM") as ps_tr,
            tc.tile_pool(name="ps_hd", bufs=1, space="PSUM") as ps_hd,
        ):
            # ---- constants ----
            iota_t = constp.tile([TILE, OH_BATCH, TILE], BF16)
            ident = constp.tile([TILE, TILE], BF16)
            W1_t = constp.tile([TILE, 4, HID], BF16)
            W2_t = constp.tile([TILE, 2, HID], BF16)
            fc1W_t = constp.tile([TILE, 2, 32], BF16)
            fc2Wd_t = constp.tile([32, 1], BF16)
            bmat_t = constp.tile([TILE, 2 * HID], BF16)
            onesm_t = constp.tile([TILE, TILE], BF16)
            fc1b_t = constp.tile([32, 1], F32)
            fc2bd_t = constp.tile([1, 1], F32)
            edst_t = edgep.tile([TILE, NGP, 1], BF16)
            eval_t = edgep.tile([TILE, NGP, 1], BF16)

            nc.sync.dma_start(out=iota_t[:], in_=iota_d[:])
            make_identity(nc, ident[:])
            for k in range(4):
                nc.sync.dma_start(out=W1_t[:, k, :], in_=W1_d[k * TILE:(k + 1) * TILE, :])
            for k in range(2):
                nc.sync.dma_start(out=W2_t[:, k, :], in_=W2_d[k * TILE:(k + 1) * TILE, :])
                nc.sync.dma_start(out=fc1W_t[:, k, :], in_=fc1W_d[k * TILE:(k + 1) * TILE, :])
            nc.sync.dma_start(out=fc2Wd_t[:], in_=fc2Wd_d[:])
            nc.sync.dma_start(out=bmat_t[:], in_=bmat_d[:])
            nc.sync.dma_start(out=onesm_t[:], in_=onesm_d[:])
            nc.sync.dma_start(out=fc1b_t[:], in_=fc1b_d[:])
            nc.sync.dma_start(out=fc2bd_t[:], in_=fc2bd_d[:])
            nc.sync.dma_start(out=edst_t[:, :, 0], in_=edst_d[:])
            nc.sync.dma_start(out=eval_t[:, :, 0], in_=eval_d[:])
            b1row = bmat_t[:, 0:HID]
            b2row = bmat_t[:, HID:2 * HID]

            # pre-zero the gather buffers: pad slots (idx 0, val 0) write
            # real rows, but short-stripped groups never write, and virgin
            # SBUF is not guaranteed finite bf16.
            for _ in range(GATH_BUFS):
                zt = gathp.tile([TILE, CMAX, HID], BF16, tag="gath")
                nc.vector.memset(zt[:], 0.0)

            MMS = 8                    # tiles per xT strip

            def mm_slab(W_tile, nk, src_dram, src_cols, ag_in):
                """support = srcT @ W per dest tile, cast bf16, DMA to ag_in.

                src_dram: [nk*TILE, src_cols] bf16 (feature-major), streamed
                in 8-tile strips (big contiguous DMA segments) so the matmul
                starts quickly without a bulk preload.
                """
                for t0 in range(0, NT, MMS):
                    t1 = min(t0 + MMS, NT)
                    w = min(SLAB, t1 * TILE) - t0 * TILE
                    xs = xsp.tile([TILE, nk, MMS * TILE], BF16, tag="xs")
                    for k in range(nk):
                        nc.sync.dma_start(
                            out=xs[:, k, :w],
                            in_=src_dram[k * TILE:(k + 1) * TILE,
                                         t0 * TILE:t0 * TILE + w])
                    for t in range(t0, t1):
                        rt = _rows_of_tile(t)
                        co = (t - t0) * TILE
                        ps = ps_mm.tile([TILE, HID], F32, tag="mm")
                        for k in range(nk):
                            nc.tensor.matmul(
                                out=ps[:rt, :],
                                lhsT=xs[:, k, co:co + rt],
                                rhs=W_tile[:, k, :],
                                start=(k == 0), stop=(k == nk - 1),
                            )
                        ev = smallp.tile([TILE, HID], BF16, tag="mmev")
                        nc.scalar.activation(ev[:rt, :], ps[:rt, :], ACT.Copy)
                        nc.sync.dma_start(out=ag_in[t * TILE:t * TILE + rt, :],
                                          in_=ev[:rt, :])

            def segsum(table, brow_b, evac):
                """SpMM via queue-parallel gathers + one-hot matmuls.

                evac(t, psum_ap) consumes each finished dest tile.
                One gather call covers tile pair (2p, 2p+1) x chunk ch.
                One-hot tiles are built in batches of OH_BATCH groups on
                the DVE from broadcast compares against iota.
                """
                qn = 0
                oh_tiles = {}          # batch index -> tile
                next_batch = [0]

                def oh_of(g):
                    bi = g // OH_BATCH
                    while next_batch[0] <= bi:
                        k = next_batch[0]
                        bcd = ohp.tile([TILE, OH_BATCH, TILE], BF16, tag="bcd", bufs=2)
                        nc.scalar.activation(
                            bcd[:],
                            edst_t[:, k * OH_BATCH:(k + 1) * OH_BATCH, :]
                                .to_broadcast([TILE, OH_BATCH, TILE]),
                            ACT.Identity)
                        bcv = ohp.tile([TILE, OH_BATCH, TILE], BF16, tag="bcv", bufs=2)
                        nc.scalar.activation(
                            bcv[:],
                            eval_t[:, k * OH_BATCH:(k + 1) * OH_BATCH, :]
                                .to_broadcast([TILE, OH_BATCH, TILE]),
                            ACT.Identity)
                        ohb = ohp.tile([TILE, OH_BATCH, TILE], BF16, tag="oh")
                        nc.vector.tensor_tensor(
                            out=ohb[:], in0=iota_t[:], in1=bcd[:],
                            op=mybir.AluOpType.is_equal)
                        nc.vector.tensor_tensor(
                            out=ohb[:], in0=ohb[:], in1=bcv[:],
                            op=mybir.AluOpType.mult)
                        oh_tiles[k] = ohb
                        if k - 4 in oh_tiles:
                            del oh_tiles[k - 4]
                        next_batch[0] += 1
                    return oh_tiles[bi][:, g % OH_BATCH, :]

                ps_of = {}
                for (c0, c1, s0, nsl) in pieces:
                    ip = eidxp.tile([TILE, PIECE_SLOTS // 16], I16, tag="eidx")
                    nc.sync.dma_start(out=ip[:, :nsl // 16],
                                      in_=eidx_d[:, s0 // 16:(s0 + nsl) // 16])
                    for c in range(c0, c1):
                        pair, ch = c // N_CH, c % N_CH
                        ga, gb = G[2 * c], G[2 * c + 1]
                        n_idx = (ga + gb) * TILE
                        co = offs[2 * c]
                        gt = gathp.tile([TILE, CMAX, HID], BF16, tag="gath")
                        nc.gpsimd.dma_gather(
                            gt[:, :ga + gb, :],
                            table[ch * CH_ROWS:(ch + 1) * CH_ROWS, :],
                            ip[:, (co - s0) // 16:(co - s0 + n_idx) // 16],
                            n_idx, n_idx, HID,
                            single_packet=False,
                            queue_num=qn,
                        )
                        qn = (qn + 1) % N_QUEUES
                        for half, gn in ((0, ga), (1, gb)):
                            t = 2 * pair + half
                            rt = _rows_of_tile(t)
                            if ch == 0:
                                ps = ps_seg.tile([TILE, HID], F32, tag="seg")
                                ps_of[t] = ps
                                # bias fold: row 0 of onesm^T is ones
                                nc.tensor.matmul(out=ps[:rt, :],
                                                 lhsT=onesm_t[:, :rt],
                                                 rhs=brow_b,
                                                 start=True, stop=False)
                            ps = ps_of[t]
                            g0 = offs[2 * c + half] // TILE
                            for g in range(gn):
                                nc.tensor.matmul(
                                    out=ps[:rt, :],
                                    lhsT=oh_of(g0 + g)[:, :rt],
                                    rhs=gt[:, (half * ga) + g, :],
                                    start=False,
                                    stop=(ch == N_CH - 1 and g == gn - 1),
                                )
                            if ch == N_CH - 1:
                                evac(t, ps)
                                del ps_of[t]

            # ---- phase A: mm1 (xT streamed per tile) ----
            mm_slab(W1_t, 4, xT_d, SLAB, ag1_in)

            if with_collectives:
                nc.gpsimd.collective_compute(
                    "AllGather", mybir.AluOpType.bypass,
                    ins=[ag1_in[:]], outs=[ag1_out[:]], replica_groups=rep)

            # ---- phase C: L1 SpMM -> relu -> transpose -> mm2 (fused) ----
            def evac1(t, ps):
                rt = _rows_of_tile(t)
                hs = smallp.tile([TILE, HID], BF16, tag="hs")
                nc.scalar.activation(hs[:rt, :], ps[:rt, :], ACT.Relu)
                hT = smallp.tile([TILE, 2, TILE], BF16, tag="hT")
                for k in range(2):
                    tp = ps_tr.tile([TILE, TILE], BF16, tag="tr")
                    nc.tensor.transpose(
                        out=tp[:, :rt],
                        in_=hs[:rt, k * TILE:(k + 1) * TILE],
                        identity=ident[:rt, :rt])
                    nc.scalar.activation(hT[:, k, :rt], tp[:, :rt], ACT.Copy)
                ps2 = ps_mm.tile([TILE, HID], F32, tag="mm")
                for k in range(2):
                    nc.tensor.matmul(out=ps2[:rt, :], lhsT=hT[:, k, :rt],
                                     rhs=W2_t[:, k, :],
                                     start=(k == 0), stop=(k == 1))
                ev = smallp.tile([TILE, HID], BF16, tag="mmev")
                nc.scalar.activation(ev[:rt, :], ps2[:rt, :], ACT.Copy)
                nc.sync.dma_start(out=ag2_in[t * TILE:t * TILE + rt, :],
                                  in_=ev[:rt, :])

            segsum(ag1_out, b1row, evac1)

            if with_collectives:
                nc.gpsimd.collective_compute(
                    "AllGather", mybir.AluOpType.bypass,
                    ins=[ag2_in[:]], outs=[ag2_out[:]], replica_groups=rep)

            # ---- phase E: L2 SpMM -> logitsT -> head (fused strips) ----
            lgT = bigp.tile([TILE, 2 * SLAB], BF16, tag="big")

            def head_strip(t0):
                t1 = min(t0 + HEAD_TILES, NT)
                w = (t1 * TILE if t1 < NT else SLAB) - t0 * TILE
                dv = stagep.tile([1, HEAD_TILES * TILE], F32, tag="dv", bufs=1)
                for t in range(t0, t1):
                    rt = _rows_of_tile(t)
                    hp = ps_hd.tile([32, TILE], F32, tag="h1")
                    for k in range(2):
                        nc.tensor.matmul(out=hp[:, :rt], lhsT=fc1W_t[:, k, :],
                                         rhs=lgT[:, k * SLAB + t * TILE:k * SLAB + t * TILE + rt],
                                         start=(k == 0), stop=(k == 1))
                    haT = smallp.tile([32, TILE], BF16, tag="haT")
                    nc.scalar.activation(haT[:, :rt], hp[:, :rt], ACT.Relu,
                                         bias=fc1b_t[:])
                    zp = ps_hd.tile([1, TILE], F32, tag="h2")
                    nc.tensor.matmul(out=zp[:, :rt], lhsT=fc2Wd_t[:],
                                     rhs=haT[:, :rt], start=True, stop=True)
                    co = (t - t0) * TILE
                    nc.scalar.activation(dv[:, co:co + rt], zp[:, :rt],
                                         ACT.Identity, bias=fc2bd_t[:])
                # log_softmax over 2 classes (stable; Abs/Exp/Ln/Relu share
                # one ACT table): a = ln(1+exp(-|d|));
                # out0 = -relu(d) - a; out1 = out0 + d.
                ab = stagep.tile([1, HEAD_TILES * TILE], F32, tag="ab", bufs=1)
                nc.scalar.activation(ab[:, :w], dv[:, :w], ACT.Abs)
                ex = stagep.tile([1, HEAD_TILES * TILE], F32, tag="ex", bufs=1)
                nc.scalar.activation(ex[:, :w], ab[:, :w], ACT.Exp, scale=-1.0)
                nc.scalar.activation(ab[:, :w], ex[:, :w], ACT.Ln, bias=1.0)
                rl = stagep.tile([1, HEAD_TILES * TILE], F32, tag="rl", bufs=1)
                nc.scalar.activation(rl[:, :w], dv[:, :w], ACT.Relu)
                zo0 = stagep.tile([1, HEAD_TILES * TILE], F32, tag="zo0", bufs=1)
                nc.vector.scalar_tensor_tensor(
                    out=zo0[:, :w], in0=rl[:, :w], scalar=-1.0, in1=ab[:, :w],
                    op0=mybir.AluOpType.mult, op1=mybir.AluOpType.subtract)
                zo1 = stagep.tile([1, HEAD_TILES * TILE], F32, tag="zo1", bufs=1)
                nc.vector.tensor_tensor(
                    out=zo1[:, :w], in0=zo0[:, :w], in1=dv[:, :w],
                    op=mybir.AluOpType.add)
                nc.sync.dma_start(out=out_d[0:1, t0 * TILE:t0 * TILE + w],
                                  in_=zo0[:, :w])
                nc.sync.dma_start(out=out_d[1:2, t0 * TILE:t0 * TILE + w],
                                  in_=zo1[:, :w])

            def evac2(t, ps):
                rt = _rows_of_tile(t)
                ev = smallp.tile([TILE, HID], BF16, tag="ev")
                nc.scalar.activation(ev[:rt, :], ps[:rt, :], ACT.Copy)
                for k in range(2):
                    tp = ps_tr.tile([TILE, TILE], BF16, tag="tr")
                    nc.tensor.transpose(
                        out=tp[:, :rt],
                        in_=ev[:rt, k * TILE:(k + 1) * TILE],
                        identity=ident[:rt, :rt])
                    nc.scalar.activation(
                        lgT[:, k * SLAB + t * TILE:k * SLAB + t * TILE + rt],
                        tp[:, :rt], ACT.Copy)
                if t % HEAD_TILES == HEAD_TILES - 1 or t == NT - 1:
                    head_strip(t - t % HEAD_TILES)

            segsum(ag2_out, b2row, evac2)

    nc.compile()
    return nc


def make_in_maps(inputs, per_core, S):
    """Build per-core input maps from full problem inputs."""
    NG = S // TILE
    NGP = math.ceil(NG / OH_BATCH) * OH_BATCH
    x = np.asarray(inputs["inputs"], np.float32)
    W1 = np.ascontiguousarray(np.asarray(inputs["W1"], np.float32).astype(BF))
    W2 = np.ascontiguousarray(np.asarray(inputs["W2"], np.float32).astype(BF))
    fc1W = np.ascontiguousarray(np.asarray(inputs["fc1_W"], np.float32).astype(BF))
    fc2W = np.asarray(inputs["fc2_W"], np.float32)
    fc2Wd = np.ascontiguousarray((fc2W[:, 1] - fc2W[:, 0]).reshape(32, 1)).astype(BF)
    bmat = np.zeros((TILE, 2 * HID), np.float32)
    bmat[0, :HID] = np.asarray(inputs["b1"], np.float32)
    bmat[0, HID:] = np.asarray(inputs["b2"], np.float32)
    bmat = bmat.astype(BF)
    onesm = np.zeros((TILE, TILE), np.float32)
    onesm[0, :] = 1.0
    onesm = onesm.astype(BF)
    fc1b = np.asarray(inputs["fc1_b"], np.float32).reshape(32, 1)
    fc2b = np.asarray(inputs["fc2_b"], np.float32)
    fc2bd = np.array([[fc2b[1] - fc2b[0]]], np.float32)
    iota = np.tile(np.arange(TILE, dtype=np.float32),
                   (TILE, OH_BATCH, 1)).astype(BF)

    in_maps = []
    for p in range(N_CORES):
        xT = np.ascontiguousarray(x[p * SLAB:(p + 1) * SLAB, :].T.astype(BF))
        edst = np.zeros((TILE, NGP), np.float32)
        edst[:, :NG] = per_core[p]["edst"]
        edst = edst.astype(BF)
        evalv = np.zeros((TILE, NGP), np.float32)
        evalv[:, :NG] = per_core[p]["eval"]
        evalv = evalv.astype(BF)
        m = {
            "xT": xT, "W1": W1, "W2": W2, "fc1W": fc1W, "fc2Wd": fc2Wd,
            "bmat": bmat, "onesm": onesm, "fc1b": fc1b, "fc2bd": fc2bd,
            "iota": iota,
            "eidx": per_core[p]["eidx"],
            "edst": np.ascontiguousarray(edst),
            "eval": np.ascontiguousarray(evalv),
        }
        in_maps.append(m)
    return in_maps


LAST_RESULT = None


def kernel(**inputs):
    global LAST_RESULT
    G, offs, S, per_core = preprocess_edges(
        inputs["edge_rows"], inputs["edge_cols"], inputs["edge_vals"])
    nc = build_program(G, offs, S, with_collectives=True)
    in_maps = make_in_maps(inputs, per_core, S)
    res = run_bass_kernel_spmd(nc, in_maps, list(range(N_CORES)))
    LAST_RESULT = res
    out = np.concatenate([res.results[p]["out"].T for p in range(N_CORES)], axis=0)
    return np.ascontiguousarray(out.astype(np.float32))
